# revision 15
# baseline (speedup 1.0000x reference)
"""Trainium2 Bass kernel: pointwise-conv (GEMM) + BatchNorm (folded) + LIF scan
+ spike-rate mean, sharded over 8 NeuronCores by TIME chunks.

Reference semantics (fp32):
    y   = einsum('bct,oc->bot', x, W)                   # [B, Cout, T]
    yb  = (y - mean) * (rsqrt(var+eps) * gamma) + beta  # BN (inference)
    v' = v + (yb_t - v)/2 ; s = (v' >= 1) ; v'' = v' * (1-s)   # LIF, T steps
    out = mean_t(s)                                     # [B, Cout]

Key facts exploited:
  * BN + the 1/TAU charge factor fold into the conv weights on the host:
        z_t = (0.5*gamma*rsqrt(var+eps) * W) @ x_t + bias
    and the LIF step becomes  u = 0.5*v + z ; spike = u>=1 ; v = u*(u<1).
  * The LIF recurrence forgets its state at rate 0.5/step, so a time chunk
    is computed exactly from a zero state started WARM=16 steps earlier
    (state influence 0.5^16 ~ 1.5e-5; measured: no additional absmax error).
    Time-sharding is embarrassingly parallel with a WARM-step overlap.
  * fp32 matmuls run at 1/4 PE rate; instead both operands split into
    bf16 hi+lo and 3 bf16 matmuls (hi*hi + hi*lo + lo*hi) run at full PE
    rate with fp32 PSUM accumulation (residual ~2^-18 relative; same
    single spike-flip count as exact fp32).  f32r was measured at 1.85e-4
    relative error on this HW - unusable here.
  * Warmup windows need only the hi*hi term (WARM_TERMS=1), so their x is
    shipped hi-only (warm_hi): halves the warm-window DMA bytes.
  * LIF scan: ONE custom DVE op per step (u_t = select(u_prev < 1,
    0.5*u_prev + z_t, z_t)), and spike counting as ONE bulk
    scalar_tensor_tensor per owned window (acc_win += (u >= 1) over
    [128, ts, 2, B]) with a log2(ts) fold at the end (COUNT_ENGINE
    "window").  The u_out export variant was 8 MB/core of extra HBM
    traffic and measurably slower.

Measured on this trn2 (paired For_i(9)-vs-For_i(137) delta timing, which
cancels the ~85 ms axon dispatch overhead AND its multi-ms drift):
  * MM+LDW chain, N=512 bf16: ~225 ns/MM -> 400-MM GEMM alone ~90 us.
  * x DMA 17.8 MB/core alone ~57 us; DMA+scan+evac (no MM) ~63 us.
  * Concurrent DMA degrades the PE stream (SBUF write-vs-read adjacency):
    independent streams overlap perfectly, but a consumed x stream costs
    ~+15-25 us over max(GEMM, DMA).  Mitigations measured best:
    9-deep x window rotation (72 KB; 18-deep reintroduces address-adjacent
    conflicts, half-iteration mega-DMAs are far worse), merged PSUM
    evacuation (1 ACT op/window), hi-only warm windows.
  * End-to-end: 142 us (prev session export baseline) -> ~122 us
    (window-count re-measure) -> ~109-111 us with this configuration.
"""

import sys
import numpy as np

if "/opt/trn_rl_repo" not in sys.path:
    sys.path.insert(0, "/opt/trn_rl_repo")

# --- problem constants (hardcoded; kernel.py must be self-contained) ---
B, CIN, T, COUT = 64, 512, 1024, 256
NCORES = 8
WARM = 16                    # warmup steps per core (state influence 0.5^16
                             # ~ 1.5e-5; flips need |u-1| < ~1e-5 * 0.5^k at
                             # owned step k -- measured 0 extra absmax error)
TCH = T // NCORES            # 128 owned steps / core
TLOC = WARM + TCH            # 144 local steps
TS = 8                       # time-steps per matmul/psum window
NTS = TLOC // TS             # 18 windows
KO = CIN // 128              # 4 contraction chunks
BN_EPS = 1e-5

_CACHE = {}

# "bf16x3": 3 bf16 hi/lo-split matmuls at full PE rate (default)
# "f32"   : exact fp32 matmuls (4 cycles/row on PE)
MM_MODE = "bf16x3"
# engine that accumulates the spike count:
#   "vector"     - 3rd fused STT op per step on DVE
#   "pool_block" - per-16-step block on GPSIMD: is_ge mask + tree-add
#                  (keeps DVE at 2 ops/step; Pool is otherwise idle)
#   "sign_tree"  - ACT Sign(u-1) per owned window (exact +-1/0), GPSIMD
#                  pairwise tree-add over the window + acc.  Host recovers
#                  count = (sum_sign + owned)/2.  DVE stays at 2 ops/step.
#   "window"     - ONE bulk STT per owned window: acc_win[t,ch,b] +=
#                  (u[t,ch,b] >= 1) over the whole [128, ts, 2, B] block,
#                  plus a log2(ts)-op fold at the end of the body.  Avoids
#                  the per-step serial acc chain entirely.
#   "window_pool"- same but the bulk STT + fold run on GPSIMD.
#   "export"     - DMA each owned window's u block to DRAM; the host
#                  thresholds u >= 1 and sums (bit-exact, zero engine
#                  cost beyond the outbound DMA).
COUNT_ENGINE = "window"
# matmul terms used for windows fully inside the warmup: the lo-term
# corrections (~2^-9 of z) only matter within ~1e-5 of the firing
# threshold, and warmup state influence on owned spikes is itself
# attenuated by >=0.5^k; measured zero extra flips with hi-only warmup.
WARM_TERMS = 1
# "stock": charge + reset as two scalar_tensor_tensor ops per step.
# "fused": one custom DVE op per step -- u_t = select(u_{t-1} < 1,
#          0.5*u_{t-1} + z_t, z_t) -- which halves the serial DVE
#          dependency chain (the end-to-end bottleneck) and keeps u_t
#          materialized in SBUF for the sign-tree count.
SCAN_MODE = "fused"

_LIF_OP = None


def _get_lif_op():
    """Register (once) and return the fused LIF-step custom DVE op.

    Registration is process-local: appends to concourse.dve_ops.OPS and
    the sub-opcode map, so the per-NEFF DVE table generator and CoreSim
    reference path both see it.  The uops sha is computed at definition
    time (self-pinned).
    """
    global _LIF_OP
    if _LIF_OP is not None:
        return _LIF_OP
    import numpy as np
    from concourse import dve_ops
    from concourse.dve_ops import DveOp, OPS, CUSTOM_DVE_SPECS
    from concourse.dve_spec import (Spec, Src0, Src1, C0, C1, select, lower,
                                    _has_src1)
    from concourse.dve_uop import DveOpSpec

    name = "LIF_STEP_ANT"
    for existing in OPS:
        if existing.name == name:
            _LIF_OP = existing
            return _LIF_OP

    u = Src0 * C0 + Src1
    body = select(Src0 < C1, u, Src1)
    spec = Spec(
        body=body,
        reference=lambda in0, in1, s0, s1, imm2: np.where(
            in0 < s1, (in0 * s0 + in1).astype(np.float32), in1
        ).astype(np.float32),
    )
    shas = {}
    for ver in ("v3", "v4"):
        try:
            tmp = DveOpSpec(name=name, opcode=None,
                            uops=lower(spec, ver=ver),
                            rd1_en=_has_src1(spec))
            shas[ver] = tmp.sha(ver)
        except Exception:
            pass
    lif = DveOp(name, spec, subdim=False, uops_sha=shas)
    OPS.append(lif)
    CUSTOM_DVE_SPECS[name] = spec
    dve_ops._SUB_OPCODE_FOR_NAME[name] = (
        max(dve_ops._SUB_OPCODE_FOR_NAME.values()) + 1)
    assert dve_ops._SUB_OPCODE_FOR_NAME[name] < 0x20
    _LIF_OP = lif
    return _LIF_OP


def _build_nc(with_bias: bool, mm_mode: str = MM_MODE,
              count_engine: str = COUNT_ENGINE, reps: int = 1,
              loop_reps: int = 0, warm: int = WARM,
              scan_ops: int = 3, mm_terms: int = 0,
              skip_mm: bool = False, skip_evac: bool = False,
              ts: int = TS, bufs: tuple = (9, 3, 4), taper: bool = False,
              fuse_dma: bool = True, evac_merge: bool = True,
              scan_mode: str = SCAN_MODE, dma_rings: int = 1,
              out_ring: str = "sync", xsep: int = 0,
              warm_full: bool = False, psum_split: int = 0,
              static_x: bool = False, halves: int = 0,
              warm_hi: bool = True):
    import concourse.tile as tile
    from concourse import bacc, mybir

    f32 = mybir.dt.float32
    bf16 = mybir.dt.bfloat16
    op = mybir.AluOpType
    split = mm_mode == "bf16x3"
    x_dt = bf16 if split else f32
    nhl = 2 if split else 1

    nc = bacc.Bacc(None)
    # per-core inputs, host-prearranged so every DMA is one contiguous block:
    #   xk [KO, 128, NTS, nhl, B, TS]  (nhl=2: bf16 hi/lo split of x)
    #   wT [nhl, CIN, COUT]            (folded weights, k-major)
    if taper:
        # per-window contiguous blocks, concatenated along the free axis
        xk = nc.declare_dram_parameter(
            "xk", [128, KO * nhl * B * (warm + TCH)], x_dt, isOutput=False)
    elif fuse_dma:
        # all KO chunks of a window in one contiguous 8KB/partition block
        nts = (warm + TCH) // ts
        nwarm_win = warm // ts
        if warm_hi and split:
            xk = nc.declare_dram_parameter(
                "xk", [128, nts - nwarm_win, KO, nhl, B, ts], x_dt,
                isOutput=False)
            xkw = nc.declare_dram_parameter(
                "xkw", [128, nwarm_win, KO, 1, B, ts], x_dt, isOutput=False)
        else:
            warm_hi = False
            xk = nc.declare_dram_parameter(
                "xk", [128, nts, KO, nhl, B, ts], x_dt, isOutput=False)
    else:
        nts = (warm + TCH) // ts
        xk = nc.declare_dram_parameter("xk", [KO, 128, nts, nhl, B, ts], x_dt,
                                       isOutput=False)
    wT = nc.declare_dram_parameter("wT", [128, nhl, KO, COUT], x_dt,
                                   isOutput=False)
    if with_bias:
        bvec = nc.declare_dram_parameter("bvec", [1, 2, 128], f32, isOutput=False)
    counts = nc.declare_dram_parameter("counts", [128, 2, B], f32, isOutput=True)
    u_out = None
    if count_engine == "export":
        now = (warm + TCH - warm) // ts  # owned windows
        u_out = nc.declare_dram_parameter(
            "u_out", [128, now, ts, 2, B], f32, isOutput=True)

    with tile.TileContext(nc) as tc:
        with (
            tc.tile_pool(name="consts", bufs=1) as consts,
            tc.tile_pool(name="xs", bufs=1 if xsep else bufs[0]) as xs,
            tc.tile_pool(name="xw", bufs=2) as xw_pool,
            tc.tile_pool(name="spc", bufs=1) as spc,
            tc.tile_pool(name="xs2", bufs=1) as xs2,
            tc.tile_pool(name="zs", bufs=bufs[1]) as zs,
            tc.tile_pool(name="psum", bufs=bufs[2], space="PSUM") as psum,
        ):
            if xsep:
                # dead spacer tile: forces xs and xs2 arenas xsep KB apart so
                # DMA writes never land adjacent to the tile PE is reading
                _pad = spc.tile([128, xsep * 1024], mybir.dt.uint8, tag="pad")
            xs = (xs, xs2) if xsep else xs
            # folded weights: [ki, hl, ko, m] with m = ch*128 + mi
            w_sb = consts.tile([128, nhl, KO, COUT], x_dt)
            nc.scalar.dma_start(w_sb, wT[:])

            bias_sb = ones_sb = None
            if with_bias:
                bias_sb = consts.tile([1, 2, 128], f32)
                nc.sync.dma_start(bias_sb, bvec[:])
                ones_sb = consts.tile([1, min(B, 512 // ts) * ts], f32)
                nc.vector.memset(ones_sb, 1.0)

            v = consts.tile([128, 2, B], f32)
            acc = consts.tile([128, 2, B], f32)
            neg1 = None
            if count_engine == "sign_tree":
                neg1 = consts.tile([128, 1], f32)
                nc.vector.memset(neg1, -1.0)
            acc_win = None
            if count_engine in ("window", "window_pool"):
                acc_win = consts.tile([128, ts, 2, B], f32)

            # reps>1 / loop_reps>0 repeat the compute for benchmarking only
            xkw_arg = xkw if warm_hi else None
            if loop_reps > 0:
                with tc.For_i(0, loop_reps, 1):
                    _emit_body(nc, tc, xs, zs, psum, xk, counts, w_sb, v, acc,
                               bias_sb, ones_sb, split, count_engine, op, f32,
                               x_dt, mybir, warm, scan_ops, mm_terms,
                               skip_mm, skip_evac, ts, taper, fuse_dma,
                               evac_merge, neg1, scan_mode, acc_win, u_out,
                               dma_rings, out_ring, warm_full, psum_split,
                               static_x, halves, xkw_arg, xw_pool)
            else:
                for _rep in range(reps):
                    _emit_body(nc, tc, xs, zs, psum, xk, counts, w_sb, v, acc,
                               bias_sb, ones_sb, split, count_engine, op, f32,
                               x_dt, mybir, warm, scan_ops, mm_terms,
                               skip_mm, skip_evac, ts, taper, fuse_dma,
                               evac_merge, neg1, scan_mode, acc_win, u_out,
                               dma_rings, out_ring, warm_full, psum_split,
                               static_x, halves, xkw_arg, xw_pool)

    if not nc.is_finalized():
        nc.finalize()
    return nc


def _emit_body(nc, tc, xs, zs, psum, xk, counts, w_sb, v, acc,
               bias_sb, ones_sb, split, count_engine, op, f32, x_dt, mybir,
               warm=WARM, scan_ops=3, mm_terms=0,
               skip_mm=False, skip_evac=False, ts=TS, taper=False,
               fuse_dma=True, evac_merge=False, neg1=None,
               scan_mode=SCAN_MODE, acc_win=None, u_out=None,
               dma_rings=1, out_ring="sync", warm_full=False,
               psum_split=0, static_x=False, halves=0, xkw=None,
               xw_pool=None):
    with_bias = bias_sb is not None
    nhl = 2 if split else 1
    # (w_half, x_half) term list: hi*hi + hi*lo + lo*hi
    terms = [(0, 0), (0, 1), (1, 0)] if split else [(0, 0)]
    if mm_terms:
        terms = terms[:mm_terms]

    nc.vector.memset(v, 0.0)
    nc.vector.memset(acc, 0.0)
    if acc_win is not None:
        eng = nc.gpsimd if count_engine == "window_pool" else nc.vector
        eng.memset(acc_win, 0.0)

    tloc = warm + TCH
    if taper:
        # start-only taper: scan pipeline fills faster; PE extra cost sits
        # in the otherwise-idle head
        windows = [4, 4] + [8] * ((tloc - 8) // 8)
        assert sum(windows) == tloc
    else:
        windows = [ts] * (tloc // ts)
    t_base = 0
    half_tiles = {}
    if halves:
        nwin_h = len(windows) // halves
    for tsi, tsw in enumerate(windows):
        nbb = min(B, 512 // tsw)
        # ---- load x window (contig; one DMA covers all KO if fuse_dma) ----
        if xkw is not None and t_base + tsw <= warm:
            xta = xw_pool.tile([128, KO, 1, B, tsw], x_dt, tag="xaw",
                               name="xtaw")
            nc.sync.dma_start(xta, xkw[:, tsi])
            xts = [xta[:, ko] for ko in range(KO)]
        elif halves:
            h = tsi // nwin_h
            if tsi % nwin_h == 0:
                xht = xs[h % 2] if isinstance(xs, tuple) else xs
                xht = xht.tile([128, nwin_h, KO, nhl, B, tsw], x_dt,
                               tag="xhalf", name="xht")
                nc.sync.dma_start(xht, xk[:, h * nwin_h:(h + 1) * nwin_h])
                half_tiles[h] = xht
            xta = half_tiles[h][:, tsi % nwin_h]
            xts = [xta[:, ko] for ko in range(KO)]
        elif taper:
            xta = xs.tile([128, KO, nhl, B, tsw], x_dt, tag=f"xa_{tsw}")
            off = KO * nhl * B * t_base
            sz = KO * nhl * B * tsw
            nc.sync.dma_start(
                xta, xk[:, off:off + sz].rearrange(
                    "p (ko h b t) -> p ko h b t", ko=KO, h=nhl, b=B))
            xts = [xta[:, ko] for ko in range(KO)]
        elif static_x:
            if tsi == 0:
                xpool = xs[0] if isinstance(xs, tuple) else xs
                _emit_body._xstat = xpool.tile(
                    [128, KO, nhl, B, tsw], x_dt, tag="xstat", name="xstat")
                nc.vector.memset(_emit_body._xstat, 0.0)
            xta = _emit_body._xstat
            xts = [xta[:, ko] for ko in range(KO)]
        elif fuse_dma:
            if isinstance(xs, tuple):
                xpool = xs[tsi % 2]
                xta = xpool.tile([128, KO, nhl, B, tsw], x_dt, tag=f"xa_{tsw}")
            else:
                xta = xs.tile([128, KO, nhl, B, tsw], x_dt, tag=f"xa_{tsw}")
            ring = nc.sync if (dma_rings == 1 or tsi % 2 == 0) else nc.scalar
            tsi_eff = tsi - (warm // tsw if xkw is not None else 0)
            ring.dma_start(xta, xk[:, tsi_eff])
            xts = [xta[:, ko] for ko in range(KO)]
        else:
            xts = []
            for ko in range(KO):
                xt = xs.tile([128, nhl, B, tsw], x_dt, tag=f"x{ko}_{tsw}")
                nc.sync.dma_start(xt, xk[ko, :, tsi])
                xts.append(xt)

        # ---- matmul: psum[:, ch, (b,t)] += W'.T @ x  (split terms) ----
        wterms = terms
        if WARM_TERMS and t_base + tsw <= warm and not warm_full:
            wterms = terms[:WARM_TERMS]
        if psum_split:
            pts = []
            for ch in range(2):
                ptc_t = psum.tile([128, 1, B * tsw], f32, tag=f"pt{ch}",
                                  name=f"pts{ch}")
                pts.append(ptc_t)
            pt = None
        else:
            pt = psum.tile([128, 2, B * tsw], f32, tag="pt")
        for ch in range(2) if not skip_mm else ():
            n_acc = len(wterms) * KO
            i_acc = 0
            ptc = pts[ch][:, 0] if psum_split else pt[:, ch]
            for ko in range(KO):
                for (wh, xh) in wterms:
                    lhsT = w_sb[:, wh, ko, ch * 128:(ch + 1) * 128]
                    first = i_acc == 0
                    last = i_acc == n_acc - 1
                    i_acc += 1
                    for nb in range(B // nbb):
                        nc.tensor.matmul(
                            ptc[:, nb * nbb * tsw:(nb + 1) * nbb * tsw],
                            lhsT,
                            xts[ko][:, xh, nb * nbb:(nb + 1) * nbb, :],
                            start=first,
                            stop=(last and not with_bias),
                        )
            if with_bias:
                for nb in range(B // nbb):
                    nc.tensor.matmul(
                        pt[:, ch, nb * nbb * tsw:(nb + 1) * nbb * tsw],
                        bias_sb[:, ch, :],
                        ones_sb[:, :nbb * tsw],
                        start=False,
                        stop=True,
                    )

        # ---- evacuate psum -> sbuf z-block [128, TS, 2, B] (ACT) ----
        zb = zs.tile([128, tsw, 2, B], f32, tag=f"zb{tsw}")
        if not (skip_mm or skip_evac):
            if evac_merge:
                nc.scalar.copy(
                    out=zb[:],
                    in_=pt.rearrange("p c (b t) -> p t c b", t=tsw),
                )
            else:
                for ch in range(2):
                    src_pt = pts[ch][:, 0] if psum_split else pt[:, ch]
                    nc.scalar.copy(
                        out=zb[:, :, ch, :],
                        in_=src_pt.rearrange("p (b t) -> p t b", t=tsw),
                    )

        # ---- LIF scan ----
        if scan_mode == "fused" and scan_ops >= 2:
            # one custom DVE op per step: u_t = select(u_prev < 1,
            # 0.5*u_prev + z_t, z_t); zb[ti] holds z_t and is overwritten
            # in place with u_t.  u_prev for the first step of the body is
            # the zero-initialized v tile; afterwards the previous zb slot
            # (including across window boundaries -- the zs pool keeps the
            # previous buffer alive through that read).
            lif = _get_lif_op()
            for ti in range(tsw):
                t = t_base + ti
                u_prev = v if t == 0 else prev_u
                nc.vector._custom_dve(
                    lif, out=zb[:, ti], in0=u_prev, in1=zb[:, ti],
                    s0=0.5, s1=1.0,
                )
                prev_u = zb[:, ti]
                if count_engine == "vector" and t >= warm and scan_ops >= 3:
                    nc.vector.scalar_tensor_tensor(
                        out=acc, in0=zb[:, ti], scalar=1.0, in1=acc,
                        op0=op.is_ge, op1=op.add,
                    )
        else:
            for ti in range(tsw):
                t = t_base + ti
                u = zb[:, ti]  # holds z_t; overwritten in place with u_t
                if scan_ops >= 1:
                    nc.vector.scalar_tensor_tensor(
                        out=u, in0=v, scalar=0.5, in1=u,
                        op0=op.mult, op1=op.add,
                    )
                if scan_ops >= 3 and t >= warm and count_engine == "vector":
                    nc.vector.scalar_tensor_tensor(
                        out=acc, in0=u, scalar=1.0, in1=acc,
                        op0=op.is_ge, op1=op.add,
                    )
                if scan_ops >= 2:
                    nc.vector.scalar_tensor_tensor(
                        out=v, in0=u, scalar=1.0, in1=u,
                        op0=op.is_lt, op1=op.mult,
                    )

        if count_engine == "export" and t_base >= warm and scan_ops >= 2:
            nc.sync.dma_start(u_out[:, (t_base - warm) // ts], zb[:])

        if (count_engine in ("window", "window_pool") and t_base >= warm
                and scan_ops >= 2):
            eng = nc.gpsimd if count_engine == "window_pool" else nc.vector
            eng.scalar_tensor_tensor(
                out=acc_win[:, :tsw], in0=zb[:], scalar=1.0,
                in1=acc_win[:, :tsw], op0=op.is_ge, op1=op.add,
            )

        if count_engine == "sign_tree" and t_base >= warm and scan_ops >= 2:
            # zb holds u_t for the whole window; ACT computes sign(u-1)
            # (exact -1/0/+1), GPSIMD folds over t and accumulates.
            sgn = zs.tile([128, tsw, 2, B], f32, tag="sgn")
            nc.scalar.activation(out=sgn[:], in_=zb[:],
                                 func=mybir.ActivationFunctionType.Sign,
                                 bias=neg1[:])
            h = tsw
            while h > 1:
                h //= 2
                nc.gpsimd.tensor_tensor(
                    out=sgn[:, :h], in0=sgn[:, :h], in1=sgn[:, h:2 * h],
                    op=op.add,
                )
            nc.gpsimd.tensor_tensor(
                out=acc, in0=acc, in1=sgn[:, 0], op=op.add,
            )

        if count_engine == "pool_block" and t_base >= warm:
            # zb still holds all 16 u_t tiles; count spikes on GPSIMD
            mblk = zs.tile([128, tsw, 2, B], f32, tag="mblk")
            nc.gpsimd.tensor_scalar(
                out=mblk[:], in0=zb[:], scalar1=1.0, scalar2=None,
                op0=op.is_ge,
            )
            h = tsw
            while h > 1:
                h //= 2
                nc.gpsimd.tensor_tensor(
                    out=mblk[:, :h], in0=mblk[:, :h], in1=mblk[:, h:2 * h],
                    op=op.add,
                )
            nc.gpsimd.tensor_tensor(
                out=acc, in0=acc, in1=mblk[:, 0], op=op.add,
            )

        t_base += tsw

    if acc_win is not None:
        eng = nc.gpsimd if count_engine == "window_pool" else nc.vector
        h = ts
        while h > 1:
            h //= 2
            eng.tensor_tensor(out=acc_win[:, :h], in0=acc_win[:, :h],
                              in1=acc_win[:, h:2 * h], op=op.add)
        getattr(nc, out_ring).dma_start(counts[:], acc_win[:, 0])
    else:
        getattr(nc, out_ring).dma_start(counts[:], acc)


def _split_bf16(a):
    """fp32 -> (hi, lo) bf16 pair with hi + lo ~ a (error ~2^-18 relative)."""
    import ml_dtypes
    hi = a.astype(ml_dtypes.bfloat16)
    lo = (a - hi.astype(np.float32)).astype(ml_dtypes.bfloat16)
    return hi, lo


def _prep_inputs(x, W, gamma, beta, run_mean, run_var, mm_mode=None,
                 warm=WARM, ts=TS, taper=False, fuse_dma=True,
                 warm_hi=True):
    """Fold BN + 1/TAU into weights; build per-core time-sharded x layouts."""
    if mm_mode is None:
        mm_mode = MM_MODE
    tloc = warm + TCH
    nts = tloc // 4 if taper else tloc // ts
    tsz = 4 if taper else ts
    split = mm_mode == "bf16x3"
    import ml_dtypes

    inv = 1.0 / np.sqrt(run_var.astype(np.float64) + BN_EPS)
    a = (0.5 * gamma.astype(np.float64) * inv)
    Wp = (W.astype(np.float64) * a[:, None]).astype(np.float32)       # [COUT, CIN]
    bp = (0.5 * (beta.astype(np.float64)
                 - run_mean.astype(np.float64) * gamma.astype(np.float64) * inv)
          ).astype(np.float32)                                        # [COUT]
    wT = np.ascontiguousarray(Wp.T)                                   # [CIN, COUT]
    if split:
        wh, wl = _split_bf16(wT)
        wTs = np.stack([wh, wl], axis=0)                              # [2,CIN,COUT]
        xh, xl = _split_bf16(x)
        xhl = np.stack([xh, xl], axis=0)                              # [2,B,CIn,T]
    else:
        wTs = wT.reshape(1, CIN, COUT)
    # device layout [ki=128, nhl, KO, COUT] so the weight DMA is contiguous
    wTs = np.ascontiguousarray(
        wTs.reshape(wTs.shape[0], KO, 128, COUT).transpose(2, 0, 1, 3))

    in_maps = []
    for c in range(NCORES):
        t0 = c * TCH - warm
        lo = max(t0, 0)
        if split:
            xc = np.zeros((2, B, CIN, tloc), dtype=ml_dtypes.bfloat16)
            xc[:, :, :, lo - t0:] = xhl[:, :, :, lo:c * TCH + TCH]
            # [2, B, CIN, tloc] -> [KO, 128, nts, 2, B, TS]
            if taper:
                # per-window contiguous blocks concatenated on the free axis
                blocks = []
                t0 = 0
                for tsw in [4, 4] + [8] * ((tloc - 8) // 8):
                    blk = (xc[:, :, :, t0:t0 + tsw]
                           .reshape(2, B, KO, 128, tsw)
                           .transpose(3, 2, 0, 1, 4)
                           .reshape(128, -1))
                    blocks.append(blk)
                    t0 += tsw
                xkc = np.ascontiguousarray(np.concatenate(blocks, axis=1))
            elif fuse_dma:
                # [128, nts, KO, nhl, B, tsz]
                xkc = np.ascontiguousarray(
                    xc.reshape(2, B, KO, 128, nts, tsz)
                      .transpose(3, 4, 2, 0, 1, 5))
                if warm_hi:
                    nw = warm // tsz
                    xkwc = np.ascontiguousarray(xkc[:, :nw, :, :1])
                    xkc = np.ascontiguousarray(xkc[:, nw:])
            else:
                xkc = np.ascontiguousarray(
                    xc.reshape(2, B, KO, 128, nts, tsz)
                      .transpose(2, 3, 4, 0, 1, 5))
        else:
            xc = np.zeros((B, CIN, tloc), dtype=np.float32)
            xc[:, :, lo - t0:] = x[:, :, lo:c * TCH + TCH]
            xkc = np.ascontiguousarray(
                xc.reshape(B, KO, 128, nts, tsz).transpose(1, 2, 3, 0, 4)
            )[:, :, :, None]
        m = {"xk": xkc, "wT": wTs}
        if warm_hi:
            m["xkw"] = xkwc
        if np.any(bp != 0):
            m["bvec"] = np.ascontiguousarray(bp.reshape(1, 2, 128))
        in_maps.append(m)
    return in_maps, bool(np.any(bp != 0))


def _postprocess(results):
    total = np.zeros((128, 2, B), dtype=np.float64)
    for r in results:
        if COUNT_ENGINE == "export":
            u = r["u_out"]  # [128, nwin, ts, 2, B]
            c = (u >= 1.0).sum(axis=(1, 2), dtype=np.float64)
        else:
            c = r["counts"].astype(np.float64)
            if COUNT_ENGINE == "sign_tree":
                # device accumulated sum of sign(u-1) over TCH owned steps;
                # spike count = (sum_sign + TCH) / 2
                c = (c + float(TCH)) / 2.0
        total += c
    # counts[ci, ch, b] -> out[b, ch*128+ci]
    out = total.transpose(2, 1, 0).reshape(B, COUT) / float(T)
    return out.astype(np.float32)


def kernel(x, W, gamma, beta, run_mean, run_var, _trace=False):
    from concourse.bass_utils import run_bass_kernel_spmd

    x = np.asarray(x, dtype=np.float32)
    W = np.asarray(W, dtype=np.float32)
    gamma = np.asarray(gamma, dtype=np.float32)
    beta = np.asarray(beta, dtype=np.float32)
    run_mean = np.asarray(run_mean, dtype=np.float32)
    run_var = np.asarray(run_var, dtype=np.float32)

    in_maps, with_bias = _prep_inputs(x, W, gamma, beta, run_mean, run_var)
    key = ("nc", with_bias, MM_MODE, COUNT_ENGINE)
    if key not in _CACHE:
        _CACHE[key] = _build_nc(with_bias)
    nc = _CACHE[key]

    res = run_bass_kernel_spmd(
        nc, in_maps, core_ids=list(range(NCORES)), trace=_trace
    )
    out = _postprocess(res.results)
    if _trace:
        return out, res
    return out


if __name__ == "__main__":
    rng = np.random.default_rng(0)
    x = rng.standard_normal((B, CIN, T), dtype=np.float32)
    W = (rng.standard_normal((COUT, CIN), dtype=np.float32) / np.sqrt(CIN)).astype(np.float32)
    out = kernel(x, W, np.ones(COUT, np.float32), np.zeros(COUT, np.float32),
                 np.zeros(COUT, np.float32), np.ones(COUT, np.float32))
    print(out.shape, out.dtype, out[:2, :4])

